# revision 18
# baseline (speedup 1.0000x reference)
"""Multi-head attention Trainium2 kernel, tensor-parallel by heads over 8 cores.

Problem: X(4,2048,1024), 16 heads x 64 dims, fused QKV+attention+out-proj.

Sharding: core c owns qkv feature slice [c*128,(c+1)*128) = 2 heads, plus the
matching 128 rows of Wo. Each core computes a full (8192,1024) partial of the
output projection; the host sums the 8 partials and adds bo. No collectives.

Device layout notes (per core):
  XT   (1024, 8192)  = X^T, features on partitions (8 k-tiles of 128)
  QT/KT/VT (128, 2048/batch): per-core qkv dims on partitions, tokens free
  S^T  computed per 128-key tile: (128 keys, 1024 queries) so that P@V
       contracts keys on partitions with V in natural (token, dim) layout
  softmax: exp on ScalarE without max subtraction (logits ~N(0,0.33), safe);
       denominator = row 64 of the ones-augmented V matmul accumulator
  out-proj consumes A^T (dims on partitions) directly.
"""

import numpy as np

N_CORES = 8
B = 4
L = 2048
D = 1024
T = B * L          # 8192
DH = 64            # head dim
SL = D // N_CORES  # 128 per-core qkv slice = 2 heads
HPC = SL // DH     # 2 heads per core
KT_X = D // 128    # 8 feature k-tiles for projections
KT_L = L // 128    # 16 key tiles per batch
QC = L // 1024     # 2 query chunks of 1024 per batch
NB = 1024 // 512   # psum bank halves per 1024-chunk

# Matmul operand mode: "f32" (exact, 4x slower PE), "f32r" (full-rate,
# reduced-mantissa PE input), "bf16" (full-rate, 16-bit storage).
MM_MODE = "f32r"

_CACHE = {}


def _split_multiwait(nc, mybir):
    """This walrus build rejects >1 sem wait per instruction; spread extras
    onto same-engine NoOps placed immediately before the instruction."""
    n = 0
    for bb in nc.main_func.blocks:
        new = []
        changed = False
        for inst in bb.instructions:
            si = inst.sync_info
            waits = list(si.on_wait) if (si and si.on_wait) else []
            if len(waits) > 1:
                changed = True
                for w in waits[:-1]:
                    n += 1
                    new.append(mybir.InstNoOp(
                        name=f"I-wsplit-{n}", ins=[], outs=[],
                        engine=inst.engine,
                        sync_info=mybir.SyncInfo(on_wait=[w], on_update=[]),
                    ))
                si.on_wait = [waits[-1]]
            new.append(inst)
        if changed:
            bb.instructions = new
    return n


def _build_program(mode):
    import concourse.bass as bass
    import concourse.tile as tile
    from concourse import mybir

    f32 = mybir.dt.float32
    bf16 = mybir.dt.bfloat16
    f32r = mybir.dt.float32r
    # storage dtype for matmul operands: fp32r is fp32 storage that the
    # verifier requires to be produced "rounded", so declare tensors as
    # float32r end-to-end in that mode.
    st_dt = {"bf16": bf16, "f32r": f32r, "f32": f32}[mode]

    def mm(ap):
        return ap

    nc = bass.Bass("TRN2", target_bir_lowering=False)

    xt = nc.dram_tensor("xt", [D, T], st_dt, kind="ExternalInput")
    ident_d = nc.dram_tensor("ident", [128, 128], st_dt, kind="ExternalInput")
    ones_d = nc.dram_tensor("ones1", [1, DH], st_dt, kind="ExternalInput")
    vones_d = nc.dram_tensor("vones", [128, KT_L, HPC, 1], st_dt,
                             kind="ExternalInput")
    wq = nc.dram_tensor("wq", [KT_X, 128, SL], st_dt, kind="ExternalInput")
    wk = nc.dram_tensor("wk", [KT_X, 128, SL], st_dt, kind="ExternalInput")
    wv = nc.dram_tensor("wv", [KT_X, 128, SL], st_dt, kind="ExternalInput")
    wo = nc.dram_tensor("wo", [SL, D], st_dt, kind="ExternalInput")
    bqkv = nc.dram_tensor("bqkv", [3, SL], f32, kind="ExternalInput")
    y = nc.dram_tensor("y", [T, D], f32, kind="ExternalOutput")

    Exp = mybir.ActivationFunctionType.Exp
    Ln = mybir.ActivationFunctionType.Ln

    with tile.TileContext(nc) as tc:
        with (
            tc.tile_pool(name="const", bufs=1) as constp,
            tc.tile_pool(name="wpool", bufs=1) as wpool,
            tc.tile_pool(name="proj", bufs=1) as projp,
            tc.tile_pool(name="xtp", bufs=2) as xtp,
            tc.tile_pool(name="esp", bufs=3) as esp,
            tc.tile_pool(name="yp", bufs=3) as yp,
            tc.tile_pool(name="small", bufs=4) as smallp,
            tc.tile_pool(name="psum", bufs=2, space="PSUM") as psp,
            tc.tile_pool(name="psat", bufs=2, space="PSUM") as psatp,
        ):
            # ---- constants / weights resident in SBUF ----
            ident = constp.tile([128, 128], st_dt, tag="ident")
            nc.sync.dma_start(ident[:], ident_d[:])
            ones = constp.tile([1, DH], st_dt, tag="ones")
            nc.sync.dma_start(ones[:], ones_d[:])
            bias_sb = constp.tile([128, 3], f32, tag="bias")
            nc.sync.dma_start(bias_sb[:], bqkv[:].rearrange("k p -> p k"))

            w_sb = []
            for name, wd in (("wq", wq), ("wk", wk), ("wv", wv)):
                wt = wpool.tile([128, KT_X, SL], st_dt, tag=name)
                nc.sync.dma_start(wt[:], wd[:].rearrange("kt p m -> p kt m"))
                w_sb.append(wt)
            wo_sb = wpool.tile([SL, D], st_dt, tag="wo")
            nc.sync.dma_start(wo_sb[:], wo[:])

            xt_view = xt[:].rearrange("(kt p) t -> p kt t", p=128)

            for b in range(B):
                # ---- per-batch persistent tiles ----
                qt_sb = projp.tile([128, L], st_dt, tag="qt")
                kt_sb = projp.tile([128, L], st_dt, tag="kt")
                vt_sb = projp.tile([128, L], st_dt, tag="vt")
                at_sb = projp.tile([128, L], st_dt, tag="at")
                v_sb = projp.tile([128, KT_L, HPC, DH + 1], st_dt, tag="vn")

                # ---- phase A: QKV projections (transposed layout) ----
                for qc in range(QC):
                    t0 = b * L + qc * 1024
                    xt_t = xtp.tile([128, KT_X, 1024], st_dt, tag="xt")
                    nc.sync.dma_start(xt_t[:], xt_view[:, :, t0:t0 + 1024])
                    for pi, dst in ((0, qt_sb), (1, kt_sb), (2, vt_sb)):
                        ps = psp.tile([128, 1024], f32, tag="ps")
                        for kt in range(KT_X):
                            for qn in range(NB):
                                nc.tensor.matmul(
                                    ps[:, qn * 512:(qn + 1) * 512],
                                    lhsT=mm(w_sb[pi][:, kt, :]),
                                    rhs=mm(xt_t[:, kt, qn * 512:(qn + 1) * 512]),
                                    start=(kt == 0), stop=(kt == KT_X - 1),
                                )
                        nc.vector.tensor_scalar_add(
                            dst[:, qc * 1024:(qc + 1) * 1024], ps[:],
                            bias_sb[:, pi:pi + 1])

                # ---- phase B: V to natural layout (keys on partitions),
                #      with a ones column at dv index 64 ----
                nc.sync.dma_start(v_sb[:, :, :, DH:DH + 1], vones_d[:])
                for kt2 in range(KT_L):
                    pt = psp.tile([128, 1024], st_dt, tag="ps")
                    nc.tensor.transpose(
                        pt[:, 0:128], vt_sb[:, kt2 * 128:(kt2 + 1) * 128],
                        ident[:])
                    for h in range(HPC):
                        nc.vector.tensor_copy(
                            v_sb[:, kt2, h, 0:DH],
                            pt[:, h * DH:(h + 1) * DH])

                # ---- phase C: attention, flash-style over key tiles ----
                for qc in range(QC):
                    q0 = qc * 1024
                    ats = [psatp.tile([DH + 1, 1024], f32, tag="pat",
                                      name=f"at_ps{h}")
                           for h in range(HPC)]
                    for kt2 in range(KT_L):
                        for h in range(HPC):
                            ps = psp.tile([128, 1024], f32, tag="ps")
                            for qn in range(NB):
                                nc.tensor.matmul(
                                    ps[:, qn * 512:(qn + 1) * 512],
                                    lhsT=mm(kt_sb[h * DH:(h + 1) * DH,
                                                  kt2 * 128:(kt2 + 1) * 128]),
                                    rhs=mm(qt_sb[h * DH:(h + 1) * DH,
                                                 q0 + qn * 512:q0 + (qn + 1) * 512]),
                                    start=True, stop=True,
                                    tile_position=(h * DH, 0),
                                )
                            es = esp.tile([128, 1024], st_dt, tag="es")
                            nc.scalar.activation(es[:], ps[:], Exp)
                            for qn in range(NB):
                                nc.tensor.matmul(
                                    ats[h][:, qn * 512:(qn + 1) * 512],
                                    lhsT=mm(v_sb[:, kt2, h, :]),
                                    rhs=mm(es[:, qn * 512:(qn + 1) * 512]),
                                    start=(kt2 == 0), stop=(kt2 == KT_L - 1),
                                )
                    # normalize by the softmax denominator (row DH of ats)
                    for h in range(HPC):
                        # 1/sumexp on ScalarE as exp(-ln(x)): same ACT table
                        # set as the softmax exp, ~5e-5 relative accuracy.
                        lse = smallp.tile([1, 1024], f32, tag="lse")
                        nc.scalar.activation(
                            lse[:], ats[h][DH:DH + 1, :], Ln)
                        rec = smallp.tile([1, 1024], st_dt, tag="rec")
                        nc.scalar.activation(rec[:], lse[:], Exp, scale=-1.0)
                        pb = psp.tile([128, 1024], f32, tag="ps")
                        for qn in range(NB):
                            nc.tensor.matmul(
                                pb[0:DH, qn * 512:(qn + 1) * 512],
                                lhsT=ones[:],
                                rhs=rec[:, qn * 512:(qn + 1) * 512],
                                start=True, stop=True,
                            )
                        pb_sb = smallp.tile([DH, 1024], f32, tag="pbsb")
                        nc.vector.tensor_copy(pb_sb[:], pb[0:DH, :])
                        nc.vector.tensor_mul(
                            at_sb[h * DH:(h + 1) * DH, q0:q0 + 1024],
                            ats[h][0:DH, :], pb_sb[:])

                # ---- phase D: partial output projection ----
                for tt in range(L // 128):
                    ps = psp.tile([128, 1024], f32, tag="ps")
                    for nn_ in range(NB):
                        nc.tensor.matmul(
                            ps[:, nn_ * 512:(nn_ + 1) * 512],
                            lhsT=mm(at_sb[:, tt * 128:(tt + 1) * 128]),
                            rhs=mm(wo_sb[:, nn_ * 512:(nn_ + 1) * 512]),
                            start=True, stop=True,
                        )
                    yt = yp.tile([128, 1024], f32, tag="y")
                    nc.vector.tensor_copy(yt[:], ps[:])
                    nc.sync.dma_start(
                        y[b * L + tt * 128:b * L + (tt + 1) * 128, :], yt[:])

    _split_multiwait(nc, mybir)
    return nc


def _get_program(mode):
    if mode not in _CACHE:
        _CACHE[mode] = _build_program(mode)
    return _CACHE[mode]


def _np_dt(mode):
    if mode == "bf16":
        import ml_dtypes
        return ml_dtypes.bfloat16
    return np.float32


def kernel(X, Wq, bq, Wk, bk, Wv, bv, Wo, bo, trace=False):
    from concourse.bass_utils import run_bass_kernel_spmd

    mode = MM_MODE
    ndt = _np_dt(mode)
    X = np.asarray(X, dtype=np.float32)
    scale = 1.0 / np.sqrt(DH)

    xt_np = np.ascontiguousarray(
        X.reshape(T, D).T).astype(ndt)                      # (D, T)
    Wq = np.asarray(Wq, np.float32) * scale
    bq_s = np.asarray(bq, np.float32) * scale
    Wk = np.asarray(Wk, np.float32)
    Wv = np.asarray(Wv, np.float32)
    Wo_ = np.asarray(Wo, np.float32)

    in_maps = []
    for c in range(N_CORES):
        sl = slice(c * SL, (c + 1) * SL)
        in_maps.append({
            "xt": xt_np,
            "ident": np.eye(128, dtype=np.float32).astype(ndt),
            "ones1": np.ones((1, DH), ndt),
            "vones": np.ones((128, KT_L, HPC, 1), ndt),
            "wq": np.ascontiguousarray(Wq[:, sl]).reshape(KT_X, 128, SL).astype(ndt),
            "wk": np.ascontiguousarray(Wk[:, sl]).reshape(KT_X, 128, SL).astype(ndt),
            "wv": np.ascontiguousarray(Wv[:, sl]).reshape(KT_X, 128, SL).astype(ndt),
            "wo": np.ascontiguousarray(Wo_[sl, :]).astype(ndt),
            "bqkv": np.stack([bq_s[sl], np.asarray(bk, np.float32)[sl],
                              np.asarray(bv, np.float32)[sl]]).astype(np.float32),
        })

    nc = _get_program(mode)
    res = run_bass_kernel_spmd(nc, in_maps, list(range(N_CORES)), trace=trace)
    kernel.last_exec_time_ns = res.exec_time_ns
    kernel.last_result = res

    out = np.zeros((T, D), np.float64)
    for c in range(N_CORES):
        out += res.results[c]["y"].astype(np.float64)
    out += np.asarray(bo, np.float64)
    return out.astype(np.float32).reshape(B, L, D)


# revision 23
# speedup vs baseline: 1.4167x; 1.4167x over previous
"""Multi-head attention Trainium2 kernel, tensor-parallel by heads over 8 cores.

Problem: X(4,2048,1024), 16 heads x 64 dims, fused QKV+attention+out-proj.

Sharding: core c owns qkv feature slice [c*128,(c+1)*128) = 2 heads, plus the
matching 128 rows of Wo. Each core computes a full (8192,1024) partial of the
output projection; the host sums the 8 partials and adds bo. No collectives.

Device layout notes (per core):
  XT   (1024, 8192)  = X^T, features on partitions (8 k-tiles of 128)
  QT/KT/VT (128, 2048/batch): per-core qkv dims on partitions, tokens free
  S^T  computed per 128-key tile: (128 keys, 1024 queries) so that P@V
       contracts keys on partitions with V in natural (token, dim) layout
  softmax: exp on ScalarE without max subtraction (logits ~N(0,0.33), safe);
       denominator = row 64 of the ones-augmented V matmul accumulator
  out-proj consumes A^T (dims on partitions) directly.
"""

import numpy as np

N_CORES = 8
B = 4
L = 2048
D = 1024
T = B * L          # 8192
DH = 64            # head dim
SL = D // N_CORES  # 128 per-core qkv slice = 2 heads
HPC = SL // DH     # 2 heads per core
KT_X = D // 128    # 8 feature k-tiles for projections
KT_L = L // 128    # 16 key tiles per batch
QC = L // 1024     # 2 query chunks of 1024 per batch
NB = 1024 // 512   # psum bank halves per 1024-chunk

# Matmul operand mode: "f32" (exact, 4x slower PE), "f32r" (full-rate,
# reduced-mantissa PE input), "bf16" (full-rate, 16-bit storage).
MM_MODE = "f32r"

_CACHE = {}


def _split_multiwait(nc, mybir):
    """This walrus build rejects >1 sem wait per instruction; spread extras
    onto same-engine NoOps placed immediately before the instruction."""
    n = 0
    for bb in nc.main_func.blocks:
        new = []
        changed = False
        for inst in bb.instructions:
            si = inst.sync_info
            waits = list(si.on_wait) if (si and si.on_wait) else []
            if len(waits) > 1:
                changed = True
                for w in waits[:-1]:
                    n += 1
                    new.append(mybir.InstNoOp(
                        name=f"I-wsplit-{n}", ins=[], outs=[],
                        engine=inst.engine,
                        sync_info=mybir.SyncInfo(on_wait=[w], on_update=[]),
                    ))
                si.on_wait = [waits[-1]]
            new.append(inst)
        if changed:
            bb.instructions = new
    return n


def _build_program(mode):
    import concourse.bass as bass
    import concourse.tile as tile
    from concourse import mybir

    f32 = mybir.dt.float32
    bf16 = mybir.dt.bfloat16
    f32r = mybir.dt.float32r
    # storage dtype for matmul operands: fp32r is fp32 storage that the
    # verifier requires to be produced "rounded", so declare tensors as
    # float32r end-to-end in that mode.
    st_dt = {"bf16": bf16, "f32r": f32r, "f32": f32}[mode]

    def mm(ap):
        return ap

    nc = bass.Bass("TRN2", target_bir_lowering=False)

    xt = nc.dram_tensor("xt", [D, T], st_dt, kind="ExternalInput")
    ident_d = nc.dram_tensor("ident", [128, 128], st_dt, kind="ExternalInput")
    ones_d = nc.dram_tensor("ones1", [1, DH], st_dt, kind="ExternalInput")
    vones_d = nc.dram_tensor("vones", [128, KT_L, HPC, 1], st_dt,
                             kind="ExternalInput")
    zeros_d = nc.dram_tensor("zeros", [DH, L], st_dt, kind="ExternalInput")
    wq = nc.dram_tensor("wq", [KT_X, 128, SL], st_dt, kind="ExternalInput")
    wk = nc.dram_tensor("wk", [KT_X, 128, SL], st_dt, kind="ExternalInput")
    wv = nc.dram_tensor("wv", [KT_X, 128, SL], st_dt, kind="ExternalInput")
    wo = nc.dram_tensor("wo", [SL, D], st_dt, kind="ExternalInput")
    bqkv = nc.dram_tensor("bqkv", [3, SL], f32, kind="ExternalInput")
    y = nc.dram_tensor("y", [T, D], f32, kind="ExternalOutput")

    Exp = mybir.ActivationFunctionType.Exp
    Ln = mybir.ActivationFunctionType.Ln

    with tile.TileContext(nc) as tc:
        with (
            tc.tile_pool(name="const", bufs=1) as constp,
            tc.tile_pool(name="wpool", bufs=1) as wpool,
            tc.tile_pool(name="proj", bufs=1) as projp,
            tc.tile_pool(name="xtp", bufs=2) as xtp,
            tc.tile_pool(name="esp", bufs=3) as esp,
            tc.tile_pool(name="yp", bufs=3) as yp,
            tc.tile_pool(name="small", bufs=4) as smallp,
            tc.tile_pool(name="psum", bufs=2, space="PSUM") as psp,
            tc.tile_pool(name="psat", bufs=2, space="PSUM") as psatp,
        ):
            # ---- constants / weights resident in SBUF ----
            ident = constp.tile([128, 128], st_dt, tag="ident")
            nc.sync.dma_start(ident[:], ident_d[:])
            ones = constp.tile([1, DH], st_dt, tag="ones")
            nc.sync.dma_start(ones[:], ones_d[:])
            bias_sb = constp.tile([128, 3], f32, tag="bias")
            nc.sync.dma_start(bias_sb[:], bqkv[:].rearrange("k p -> p k"))

            w_sb = []
            for name, wd in (("wq", wq), ("wk", wk), ("wv", wv)):
                wt = wpool.tile([128, KT_X, SL], st_dt, tag=name)
                nc.sync.dma_start(wt[:], wd[:].rearrange("kt p m -> p kt m"))
                w_sb.append(wt)
            wo_sb = wpool.tile([SL, D], st_dt, tag="wo")
            nc.sync.dma_start(wo_sb[:], wo[:])

            xt_view = xt[:].rearrange("(kt p) t -> p kt t", p=128)

            # Per-head K^T tiles, zero-padded to the full 128-partition
            # contraction: K<=64 matmuls stream at half rate on TRN2, so we
            # pad the head's 64-dim contraction with the other head's rows
            # zeroed and contract over all 128 partitions at full rate.
            kt_h = [wpool.tile([128, L], st_dt, tag=f"kth{h}", name=f"kt_h{h}")
                    for h in range(HPC)]
            nc.sync.dma_start(kt_h[0][DH:2 * DH, :], zeros_d[:])
            nc.sync.dma_start(kt_h[1][0:DH, :], zeros_d[:])

            for b in range(B):
                # ---- per-batch persistent tiles ----
                qt_sb = projp.tile([128, L], st_dt, tag="qt")
                vt_sb = projp.tile([128, L], st_dt, tag="vt")
                at_sb = projp.tile([128, L], st_dt, tag="at")
                v_sb = projp.tile([128, KT_L, HPC, DH + 1], st_dt, tag="vn")

                # ---- phase A: QKV projections (transposed layout) ----
                for qc in range(QC):
                    t0 = b * L + qc * 1024
                    xt_t = xtp.tile([128, KT_X, 1024], st_dt, tag="xt")
                    nc.sync.dma_start(xt_t[:], xt_view[:, :, t0:t0 + 1024])
                    for pi, dst in ((0, qt_sb), (1, None), (2, vt_sb)):
                        ps = psp.tile([128, 1024], f32, tag="ps")
                        for kt in range(KT_X):
                            for qn in range(NB):
                                nc.tensor.matmul(
                                    ps[:, qn * 512:(qn + 1) * 512],
                                    lhsT=mm(w_sb[pi][:, kt, :]),
                                    rhs=mm(xt_t[:, kt, qn * 512:(qn + 1) * 512]),
                                    start=(kt == 0), stop=(kt == KT_X - 1),
                                )
                        qsl = slice(qc * 1024, (qc + 1) * 1024)
                        if pi == 1:
                            # K^T goes to the per-head zero-padded tiles
                            for h in range(HPC):
                                hs = slice(h * DH, (h + 1) * DH)
                                nc.vector.tensor_scalar_add(
                                    kt_h[h][hs, qsl], ps[hs, :],
                                    bias_sb[hs, pi:pi + 1])
                        else:
                            nc.vector.tensor_scalar_add(
                                dst[:, qsl], ps[:], bias_sb[:, pi:pi + 1])

                # ---- phase B: V to natural layout (keys on partitions),
                #      with a ones column at dv index 64 ----
                nc.sync.dma_start(v_sb[:, :, :, DH:DH + 1], vones_d[:])
                for kt2 in range(KT_L):
                    pt = psp.tile([128, 1024], st_dt, tag="ps")
                    nc.tensor.transpose(
                        pt[:, 0:128], vt_sb[:, kt2 * 128:(kt2 + 1) * 128],
                        ident[:])
                    for h in range(HPC):
                        nc.vector.tensor_copy(
                            v_sb[:, kt2, h, 0:DH],
                            pt[:, h * DH:(h + 1) * DH])

                # ---- phase C: attention, flash-style over key tiles ----
                for qc in range(QC):
                    q0 = qc * 1024
                    ats = [psatp.tile([DH + 1, 1024], f32, tag="pat",
                                      name=f"at_ps{h}")
                           for h in range(HPC)]
                    for kt2 in range(KT_L):
                        for h in range(HPC):
                            ps = psp.tile([128, 1024], f32, tag="ps")
                            for qn in range(NB):
                                nc.tensor.matmul(
                                    ps[:, qn * 512:(qn + 1) * 512],
                                    lhsT=mm(kt_h[h][:, kt2 * 128:(kt2 + 1) * 128]),
                                    rhs=mm(qt_sb[:,
                                                 q0 + qn * 512:q0 + (qn + 1) * 512]),
                                    start=True, stop=True,
                                )
                            es = esp.tile([128, 1024], st_dt, tag="es")
                            nc.scalar.activation(es[:], ps[:], Exp)
                            for qn in range(NB):
                                nc.tensor.matmul(
                                    ats[h][:, qn * 512:(qn + 1) * 512],
                                    lhsT=mm(v_sb[:, kt2, h, :]),
                                    rhs=mm(es[:, qn * 512:(qn + 1) * 512]),
                                    start=(kt2 == 0), stop=(kt2 == KT_L - 1),
                                )
                    # normalize by the softmax denominator (row DH of ats)
                    for h in range(HPC):
                        # 1/sumexp on ScalarE as exp(-ln(x)): same ACT table
                        # set as the softmax exp, ~5e-5 relative accuracy.
                        lse = smallp.tile([1, 1024], f32, tag="lse")
                        nc.scalar.activation(
                            lse[:], ats[h][DH:DH + 1, :], Ln)
                        rec = smallp.tile([1, 1024], st_dt, tag="rec")
                        nc.scalar.activation(rec[:], lse[:], Exp, scale=-1.0)
                        pb = psp.tile([128, 1024], f32, tag="ps")
                        for qn in range(NB):
                            nc.tensor.matmul(
                                pb[0:DH, qn * 512:(qn + 1) * 512],
                                lhsT=ones[:],
                                rhs=rec[:, qn * 512:(qn + 1) * 512],
                                start=True, stop=True,
                            )
                        pb_sb = smallp.tile([DH, 1024], f32, tag="pbsb")
                        nc.vector.tensor_copy(pb_sb[:], pb[0:DH, :])
                        nc.vector.tensor_mul(
                            at_sb[h * DH:(h + 1) * DH, q0:q0 + 1024],
                            ats[h][0:DH, :], pb_sb[:])

                # ---- phase D: partial output projection ----
                for tt in range(L // 128):
                    ps = psp.tile([128, 1024], f32, tag="ps")
                    for nn_ in range(NB):
                        nc.tensor.matmul(
                            ps[:, nn_ * 512:(nn_ + 1) * 512],
                            lhsT=mm(at_sb[:, tt * 128:(tt + 1) * 128]),
                            rhs=mm(wo_sb[:, nn_ * 512:(nn_ + 1) * 512]),
                            start=True, stop=True,
                        )
                    yt = yp.tile([128, 1024], f32, tag="y")
                    nc.vector.tensor_copy(yt[:], ps[:])
                    nc.sync.dma_start(
                        y[b * L + tt * 128:b * L + (tt + 1) * 128, :], yt[:])

    _split_multiwait(nc, mybir)
    return nc


def _get_program(mode):
    if mode not in _CACHE:
        _CACHE[mode] = _build_program(mode)
    return _CACHE[mode]


def _np_dt(mode):
    if mode == "bf16":
        import ml_dtypes
        return ml_dtypes.bfloat16
    return np.float32


def kernel(X, Wq, bq, Wk, bk, Wv, bv, Wo, bo, trace=False):
    from concourse.bass_utils import run_bass_kernel_spmd

    mode = MM_MODE
    ndt = _np_dt(mode)
    X = np.asarray(X, dtype=np.float32)
    scale = 1.0 / np.sqrt(DH)

    xt_np = np.ascontiguousarray(
        X.reshape(T, D).T).astype(ndt)                      # (D, T)
    Wq = np.asarray(Wq, np.float32) * scale
    bq_s = np.asarray(bq, np.float32) * scale
    Wk = np.asarray(Wk, np.float32)
    Wv = np.asarray(Wv, np.float32)
    Wo_ = np.asarray(Wo, np.float32)

    in_maps = []
    for c in range(N_CORES):
        sl = slice(c * SL, (c + 1) * SL)
        in_maps.append({
            "xt": xt_np,
            "ident": np.eye(128, dtype=np.float32).astype(ndt),
            "ones1": np.ones((1, DH), ndt),
            "vones": np.ones((128, KT_L, HPC, 1), ndt),
            "zeros": np.zeros((DH, L), ndt),
            "wq": np.ascontiguousarray(Wq[:, sl]).reshape(KT_X, 128, SL).astype(ndt),
            "wk": np.ascontiguousarray(Wk[:, sl]).reshape(KT_X, 128, SL).astype(ndt),
            "wv": np.ascontiguousarray(Wv[:, sl]).reshape(KT_X, 128, SL).astype(ndt),
            "wo": np.ascontiguousarray(Wo_[sl, :]).astype(ndt),
            "bqkv": np.stack([bq_s[sl], np.asarray(bk, np.float32)[sl],
                              np.asarray(bv, np.float32)[sl]]).astype(np.float32),
        })

    nc = _get_program(mode)
    res = run_bass_kernel_spmd(nc, in_maps, list(range(N_CORES)), trace=trace)
    kernel.last_exec_time_ns = res.exec_time_ns
    kernel.last_result = res

    out = np.zeros((T, D), np.float64)
    for c in range(N_CORES):
        out += res.results[c]["y"].astype(np.float64)
    out += np.asarray(bo, np.float64)
    return out.astype(np.float32).reshape(B, L, D)
